# revision 12
# baseline (speedup 1.0000x reference)
"""Trainium2 Bass kernel for the DJconv hypergraph-propagation module.

Math (validated against the reference in an fp32 numpy prototype):
  G  = binarize(H.T @ H)            [NI, NI]   (item-item co-occurrence)
  B  = binarize(H @ G)              [NU, NI]   (jump-knowledge block; the
                                               item-side block is exactly B.T)
  user side:  dv_u = rsqrt+(rowsum H + rowsum B)
              S    = H @ (P1/colsumH) + B @ (P2/colsumB),
              P1   = H.T @ (dv_u*U),  P2 = B.T @ (dv_u*U)
              U_out = (dv_u*S + U) @ weight + bias
  item side:  dv_i = rsqrt+(colsum H + colsum B)
              T    = H.T @ (Q1/rowsumH) + B.T @ (Q2/rowsumB),
              Q1   = H @ (dv_i*I)[Sc], Q2 = B @ (dv_i*I)[Sc]
              I_out = (dv_i*T + I) @ weight + bias

Sharding over 8 cores: columns of G and B (items) are split into 8 chunks of
384; each core computes its chunk with zero communication for the heavy
phases (fp8 DoubleRow matmuls — H and the binarized intermediates are 0/1 so
fp8 is exact). Three AllReduces combine the small contractions over the item
dimension, each overlapped with independent PE work.

Hardware notes baked into the structure:
- dynamic-DMA instructions only support 2 sync waits, so the weight stream
  uses one shared pool with bufs = 16 (a multiple of the 8 HWDGE queues):
  slot reuse then lands on the same queue (FIFO, no extra wait).
- every DMA-touched SBUF tile lives in a pool opened before the stream pool
  so its address range is never reclaimed (reclaimed first-touch would fan
  in waits from many queues); compute-only phase-3 tiles go to a late pool
  that reuses the stream range.
- tiles later read by a DMA are written by a single engine (nc.vector).
"""

import numpy as np
import ml_dtypes

NU, NI, D = 4096, 3072, 64
P = 128
NCORES = 8
SC = NI // NCORES      # 384 items per core
KU = NU // P           # 32 user tiles
KI = NI // P           # 24 item tiles
SCT = SC // P          # 3 item tiles per core

AR_BF16 = True         # Q/S AllReduce payloads in bf16 (validated in proto)

_cached = {}


def _build_module():
    from contextlib import ExitStack

    import concourse.bass as bass
    import concourse.bacc as bacc
    import concourse.mybir as mybir
    import concourse.tile as tile
    from concourse.masks import make_identity

    F32 = mybir.dt.float32
    BF16 = mybir.dt.bfloat16
    FP8 = mybir.dt.float8e4
    CDT = BF16 if AR_BF16 else F32
    ALU = mybir.AluOpType
    AX = mybir.AxisListType
    DR = mybir.MatmulPerfMode.DoubleRow

    nc = bacc.Bacc("TRN2", target_bir_lowering=False)

    Hbt = nc.dram_tensor("Hbt", [KI, P, KU, P], FP8, kind="ExternalInput")
    Htbt = nc.dram_tensor("Htbt", [KU, P, KI, P], FP8, kind="ExternalInput")
    Hct = nc.dram_tensor("Hct", [P, KU, SC], FP8, kind="ExternalInput")
    Htct = nc.dram_tensor("Htct", [P, SCT, NU], BF16, kind="ExternalInput")
    Ut = nc.dram_tensor("Ut", [P, KU, D], F32, kind="ExternalInput")
    Ict = nc.dram_tensor("Ict", [P, SCT, D], F32, kind="ExternalInput")
    Wb = nc.dram_tensor("Wb", [D + 1, D], F32, kind="ExternalInput")
    Uo = nc.dram_tensor("Uo", [P, KU, D], F32, kind="ExternalOutput")
    Io = nc.dram_tensor("Io", [P, SCT, D], F32, kind="ExternalOutput")

    def inv_pos(v, out, msk, scr, rsqrt):
        # out = (v > 0) ? (rsqrt ? 1/sqrt(v) : 1/v) : 0
        nc.vector.tensor_scalar(msk, v, 0.0, None, ALU.is_gt)
        nc.vector.tensor_scalar(scr, v, 1.0, None, ALU.max)
        nc.vector.reciprocal(out, scr)
        if rsqrt:
            nc.scalar.activation(out, out, mybir.ActivationFunctionType.Sqrt)
        nc.vector.tensor_tensor(out, out, msk, ALU.mult)

    with tile.TileContext(nc) as tc, ExitStack() as ctx:
        # ---- pools whose tiles are touched by DMAs: never-reclaimed ranges
        const = ctx.enter_context(tc.tile_pool(name="const", bufs=1))
        resid = ctx.enter_context(tc.tile_pool(name="resid", bufs=1))
        vecs = ctx.enter_context(tc.tile_pool(name="vecs", bufs=1))
        comm = ctx.enter_context(tc.tile_pool(name="comm", bufs=1))
        ost = ctx.enter_context(tc.tile_pool(name="ost", bufs=2))
        dram = ctx.enter_context(tc.tile_pool(name="dram", bufs=1, space="DRAM"))

        idb = const.tile([P, P], BF16, tag="idb")
        idf = const.tile([P, P], F32, tag="idf")
        make_identity(nc, idb)
        make_identity(nc, idf)
        wb_sb = const.tile([D + 1, D], F32, tag="wb")
        nc.sync.dma_start(wb_sb, Wb[:])

        hc8 = resid.tile([P, KU, SC], FP8, tag="hc8")
        htc = resid.tile([P, SCT, NU], BF16, tag="htc")
        bsb = resid.tile([P, KU, SC], BF16, tag="bsb")
        btsb = resid.tile([P, SCT, NU], BF16, tag="btsb")
        u_sb = resid.tile([P, KU, D], F32, tag="u")
        ic_sb = resid.tile([P, SCT, D], F32, tag="ic")
        nc.sync.dma_start(hc8, Hct[:])
        nc.sync.dma_start(htc, Htct[:])
        nc.sync.dma_start(u_sb, Ut[:])
        nc.sync.dma_start(ic_sb, Ict[:])

        # degree partials that only need the inputs (run during phase G)
        rhp = vecs.tile([P, KU], F32, tag="rhp")
        ch = vecs.tile([P, SCT], F32, tag="ch")
        nc.vector.tensor_reduce(rhp, hc8, axis=AX.X, op=ALU.add)
        nc.vector.tensor_reduce(ch, htc, axis=AX.X, op=ALU.add)
        rbp = vecs.tile([P, KU], F32, tag="rbp")

        rg = [list(range(NCORES))]

        with tc.tile_pool(name="gbpool", bufs=1) as gbp:
            gb = gbp.tile([P, KI, SC], FP8, tag="gb")
            with tc.tile_pool(name="wstream", bufs=16) as wsp:
                # ------------ phase G: G[:, Sc] = binarize(H.T @ H[:, Sc]) --
                with tc.tile_pool(name="gpsum", bufs=4, space="PSUM") as gps:
                    for m in range(KI):
                        blk = wsp.tile([P, KU, P], FP8, tag="wblk")
                        nc.sync.dma_start(blk, Hbt[m])
                        ps = gps.tile([P, SC], F32, tag="gps")
                        for k in range(0, KU, 2):
                            nc.tensor.matmul(
                                ps, blk[:, k : k + 2, :], hc8[:, k : k + 2, :],
                                start=(k == 0), stop=(k == KU - 2), perf_mode=DR,
                            )
                        nc.vector.tensor_scalar(
                            gb[:, m, :], ps, 0.5, None, ALU.is_ge
                        )

                # ------------ phase B: B[:, Sc] = binarize(H @ G[:, Sc]) ----
                with (
                    tc.tile_pool(name="bpsum", bufs=2, space="PSUM") as bps,
                    tc.tile_pool(name="tpsum", bufs=4, space="PSUM") as tps,
                ):
                    for m in range(KU):
                        blk = wsp.tile([P, KU, P], FP8, tag="wblk")
                        nc.sync.dma_start(blk[:, :KI, :], Htbt[m])
                        ps = bps.tile([P, SC], F32, tag="bps")
                        for k in range(0, KI, 2):
                            nc.tensor.matmul(
                                ps, blk[:, k : k + 2, :], gb[:, k : k + 2, :],
                                start=(k == 0), stop=(k == KI - 2), perf_mode=DR,
                            )
                        # binarize + rowsum of this row-block in one DVE op
                        nc.vector.tensor_scalar(
                            bsb[:, m, :], ps, 0.5, 0.0, ALU.is_ge, ALU.add,
                            accum_out=rbp[:, m : m + 1],
                        )
                        # build B.T for this row-block while it is hot
                        for s in range(SCT):
                            tp = tps.tile([P, P], BF16, tag="tp")
                            nc.tensor.transpose(
                                tp, bsb[:, m, s * P : (s + 1) * P], idb
                            )
                            nc.any.tensor_copy(
                                btsb[:, s, m * P : (m + 1) * P], tp
                            )

        # ----- AllReduce R: rowsums of H and B (tiny, fires first) -----------
        arr_pack = vecs.tile([P, KU, 2], F32, tag="arr_pack")
        nc.vector.tensor_copy(arr_pack[:, :, 0:1], rhp[:, :, None])
        nc.vector.tensor_copy(arr_pack[:, :, 1:2], rbp[:, :, None])
        arr_in = dram.tile([P, KU, 2], F32, tag="arr_in")
        arr_out = dram.tile([P, KU, 2], F32, tag="arr_out", addr_space="Shared")
        nc.sync.dma_start(arr_in, arr_pack)
        nc.gpsimd.collective_compute(
            "AllReduce", ALU.add, replica_groups=rg,
            ins=[arr_in.opt()], outs=[arr_out.opt()],
        )
        arr_r = vecs.tile([P, KU, 2], F32, tag="arr_r")
        nc.sync.dma_start(arr_r, arr_out)
        rh = arr_r[:, :, 0]
        rb = arr_r[:, :, 1]

        # ---- late pool: compute-only tiles, may reuse the stream range -----
        late = ctx.enter_context(tc.tile_pool(name="late", bufs=1))

        # item-side dv_i (local: colsums of this chunk)
        cb = vecs.tile([P, SCT], F32, tag="cb")
        nc.vector.tensor_reduce(cb, btsb, axis=AX.X, op=ALU.add)
        dvi = vecs.tile([P, SCT], F32, tag="dvi")
        m3a = vecs.tile([P, SCT], F32, tag="m3a")
        m3b = vecs.tile([P, SCT], F32, tag="m3b")
        nc.vector.tensor_tensor(m3a, ch, cb, ALU.add)
        inv_pos(m3a, dvi, m3b, m3a, rsqrt=True)

        # Xi = dv_i * I[Sc]  (bf16 for the matmuls)
        xib = late.tile([P, SCT, D], BF16, tag="xib")
        nc.vector.tensor_tensor(
            xib, ic_sb, dvi[:, :, None].to_broadcast((P, SCT, D)), ALU.mult
        )

        # ----- Q1p = H[:,Sc] @ Xi, Q2p = B[:,Sc] @ Xi -> AllReduce Q --------
        ara = comm.tile([P, KU, 2, D], CDT, tag="ara")
        with tc.tile_pool(name="qpsum", bufs=4, space="PSUM") as qps:
            for m in range(KU):
                ms = slice(m * P, (m + 1) * P)
                ps1 = qps.tile([P, D], F32, tag="qps1")
                ps2 = qps.tile([P, D], F32, tag="qps2")
                for s in range(SCT):
                    nc.tensor.matmul(
                        ps1, htc[:, s, ms], xib[:, s, :],
                        start=(s == 0), stop=(s == SCT - 1),
                    )
                for s in range(SCT):
                    nc.tensor.matmul(
                        ps2, btsb[:, s, ms], xib[:, s, :],
                        start=(s == 0), stop=(s == SCT - 1),
                    )
                nc.vector.tensor_copy(ara[:, m, 0, :], ps1)
                nc.vector.tensor_copy(ara[:, m, 1, :], ps2)
        ara_in = dram.tile([P, KU, 2, D], CDT, tag="ara_in")
        ara_out = dram.tile([P, KU, 2, D], CDT, tag="ara_out", addr_space="Shared")
        nc.sync.dma_start(ara_in, ara)
        nc.gpsimd.collective_compute(
            "AllReduce", ALU.add, replica_groups=rg,
            ins=[ara_in.opt()], outs=[ara_out.opt()],
        )

        # ----- user side (needs only AllReduce R): P1/P2, W, partial S ------
        hcb = late.tile([P, KU, SC], BF16, tag="hcb")
        nc.any.tensor_copy(hcb, hc8)

        dvu = vecs.tile([P, KU], F32, tag="dvu")
        mku = vecs.tile([P, KU], F32, tag="mku")
        sku = vecs.tile([P, KU], F32, tag="sku")
        nc.vector.tensor_tensor(sku, rh, rb, ALU.add)
        inv_pos(sku, dvu, mku, sku, rsqrt=True)
        xub = late.tile([P, KU, D], BF16, tag="xub")
        nc.vector.tensor_tensor(
            xub, u_sb, dvu[:, :, None].to_broadcast((P, KU, D)), ALU.mult
        )

        e3 = vecs.tile([P, SCT], F32, tag="e3")
        e4 = vecs.tile([P, SCT], F32, tag="e4")
        inv_pos(ch, e3, m3b, m3a, rsqrt=False)
        inv_pos(cb, e4, m3b, m3a, rsqrt=False)

        w1b = late.tile([P, SCT, D], BF16, tag="w1b")
        w2b = late.tile([P, SCT, D], BF16, tag="w2b")
        with tc.tile_pool(name="ppsum", bufs=2, space="PSUM") as pps:
            for si in range(SCT):
                ss = slice(si * P, (si + 1) * P)
                psc = pps.tile([P, D], F32, tag="psc")
                psd = pps.tile([P, D], F32, tag="psd")
                for k in range(KU):
                    nc.tensor.matmul(
                        psc, hcb[:, k, ss], xub[:, k, :],
                        start=(k == 0), stop=(k == KU - 1),
                    )
                for k in range(KU):
                    nc.tensor.matmul(
                        psd, bsb[:, k, ss], xub[:, k, :],
                        start=(k == 0), stop=(k == KU - 1),
                    )
                # W = P / colsum directly from PSUM
                nc.vector.tensor_scalar(
                    w1b[:, si, :], psc, e3[:, si : si + 1], None, ALU.mult
                )
                nc.vector.tensor_scalar(
                    w2b[:, si, :], psd, e4[:, si : si + 1], None, ALU.mult
                )

        # partial S = H[:,Sc] @ W1 + B[:,Sc] @ W2  -> AllReduce B
        ssb = comm.tile([P, KU, D], CDT, tag="ssb")
        with tc.tile_pool(name="spsum", bufs=4, space="PSUM") as sps:
            for m in range(KU):
                ms = slice(m * P, (m + 1) * P)
                ps = sps.tile([P, D], F32, tag="sps")
                for s in range(SCT):
                    nc.tensor.matmul(
                        ps, htc[:, s, ms], w1b[:, s, :],
                        start=(s == 0), stop=False,
                    )
                for s in range(SCT):
                    nc.tensor.matmul(
                        ps, btsb[:, s, ms], w2b[:, s, :],
                        start=False, stop=(s == SCT - 1),
                    )
                nc.vector.tensor_copy(ssb[:, m, :], ps)
        arb_in = dram.tile([P, KU, D], CDT, tag="arb_in")
        arb_out = dram.tile([P, KU, D], CDT, tag="arb_out", addr_space="Shared")
        nc.sync.dma_start(arb_in, ssb)
        nc.gpsimd.collective_compute(
            "AllReduce", ALU.add, replica_groups=rg,
            ins=[arb_in.opt()], outs=[arb_out.opt()],
        )

        # ----- item side finish (needs AllReduce Q) -------------------------
        arr_q = comm.tile([P, KU, 2, D], CDT, tag="arr_q")
        nc.sync.dma_start(arr_q, ara_out)
        e1 = vecs.tile([P, KU], F32, tag="e1")
        e2 = vecs.tile([P, KU], F32, tag="e2")
        inv_pos(rh, e1, mku, sku, rsqrt=False)
        inv_pos(rb, e2, mku, sku, rsqrt=False)
        r1b = late.tile([P, KU, D], BF16, tag="r1b")
        r2b = late.tile([P, KU, D], BF16, tag="r2b")
        nc.vector.tensor_tensor(
            r1b, arr_q[:, :, 0, :], e1[:, :, None].to_broadcast((P, KU, D)), ALU.mult
        )
        nc.vector.tensor_tensor(
            r2b, arr_q[:, :, 1, :], e2[:, :, None].to_broadcast((P, KU, D)), ALU.mult
        )

        mi = late.tile([P, SCT, D], F32, tag="mi")
        with tc.tile_pool(name="mpsum", bufs=2, space="PSUM") as mps:
            for si in range(SCT):
                ss = slice(si * P, (si + 1) * P)
                psa = mps.tile([P, D], F32, tag="psa")
                psb2 = mps.tile([P, D], F32, tag="psb2")
                for k in range(KU):
                    nc.tensor.matmul(
                        psa, hcb[:, k, ss], r1b[:, k, :],
                        start=(k == 0), stop=(k == KU - 1),
                    )
                for k in range(KU):
                    nc.tensor.matmul(
                        psb2, bsb[:, k, ss], r2b[:, k, :],
                        start=(k == 0), stop=(k == KU - 1),
                    )
                nc.vector.tensor_copy(mi[:, si, :], psa)
                nc.vector.tensor_tensor(mi[:, si, :], mi[:, si, :], psb2, ALU.add)
                nc.vector.tensor_scalar(
                    mi[:, si, :], mi[:, si, :], dvi[:, si : si + 1], None, ALU.mult
                )
                nc.vector.tensor_tensor(
                    mi[:, si, :], mi[:, si, :], ic_sb[:, si, :], ALU.add
                )

        # ----- finals: out = M @ weight + bias (ones row added post-T) ------
        sfull = comm.tile([P, KU, D], CDT, tag="sfull")
        nc.sync.dma_start(sfull, arb_out)
        mu = late.tile([P, KU, D], F32, tag="mu")
        nc.vector.tensor_tensor(
            mu, sfull, dvu[:, :, None].to_broadcast((P, KU, D)), ALU.mult
        )
        nc.vector.tensor_tensor(mu, mu, u_sb, ALU.add)

        with (
            tc.tile_pool(name="fpsum", bufs=2, space="PSUM") as fps,
            tc.tile_pool(name="ftmp", bufs=2) as ftp,
        ):
            for idx in range(SCT + KU):
                if idx < SCT:
                    src = mi[:, idx, :]
                    dst = Io[:, idx, :]
                else:
                    src = mu[:, idx - SCT, :]
                    dst = Uo[:, idx - SCT, :]
                tp = fps.tile([P, P], F32, tag="ftp")
                nc.tensor.transpose(tp[:D, :], src, idf)
                mt = ftp.tile([D + 1, P], F32, tag="fmt")
                nc.vector.tensor_copy(mt[:D, :], tp[:D, :])
                nc.vector.memset(mt[D : D + 1, :], 1.0)
                po = fps.tile([P, D], F32, tag="fpo")
                nc.tensor.matmul(po, mt, wb_sb, start=True, stop=True)
                og = ost.tile([P, D], F32, tag="og")
                nc.vector.tensor_copy(og, po)
                nc.sync.dma_start(dst, og)

    nc.finalize()
    return nc


def _prep_inputs(H, U, I, weight, bias):
    import concourse.mybir as mybir

    f8 = mybir.dt.np(mybir.dt.float8e4)
    bf = ml_dtypes.bfloat16
    H8 = np.ascontiguousarray(H.astype(f8))
    H4 = H8.reshape(KU, P, KI, P)
    Hbt = np.ascontiguousarray(H4.transpose(2, 1, 0, 3))   # [KI,P,KU,P]
    Htbt = np.ascontiguousarray(H4.transpose(0, 3, 2, 1))  # [KU,P,KI,P]
    Ut = np.ascontiguousarray(
        U.astype(np.float32).reshape(KU, P, D).transpose(1, 0, 2)
    )
    wbm = np.ascontiguousarray(
        np.concatenate([weight.astype(np.float32), bias.astype(np.float32)[None, :]], 0)
    )
    Hb = H.astype(bf)
    in_maps = []
    for c in range(NCORES):
        S = slice(c * SC, (c + 1) * SC)
        Hct = np.ascontiguousarray(H8[:, S].reshape(KU, P, SC).transpose(1, 0, 2))
        Htct = np.ascontiguousarray(
            Hb[:, S].T.reshape(SCT, P, NU).transpose(1, 0, 2)
        )
        Ict = np.ascontiguousarray(
            I[S].astype(np.float32).reshape(SCT, P, D).transpose(1, 0, 2)
        )
        in_maps.append(
            {
                "Hbt": Hbt,
                "Htbt": Htbt,
                "Hct": Hct,
                "Htct": Htct,
                "Ut": Ut,
                "Ict": Ict,
                "Wb": wbm,
            }
        )
    return in_maps


def run(H, U, I, weight, bias, trace=False):
    from concourse.bass_utils import run_bass_kernel_spmd

    if "nc" not in _cached:
        _cached["nc"] = _build_module()
    nc = _cached["nc"]
    in_maps = _prep_inputs(H, U, I, weight, bias)
    res = run_bass_kernel_spmd(
        nc, in_maps, core_ids=list(range(NCORES)), trace=trace
    )
    U_out = (
        res.results[0]["Uo"].astype(np.float32).transpose(1, 0, 2).reshape(NU, D)
    )
    I_out = np.concatenate(
        [
            res.results[c]["Io"].astype(np.float32).transpose(1, 0, 2).reshape(SC, D)
            for c in range(NCORES)
        ],
        axis=0,
    )
    return (U_out, I_out), res


def kernel(H, U, I, weight, bias):
    H = np.asarray(H, dtype=np.float32)
    U = np.asarray(U, dtype=np.float32)
    I = np.asarray(I, dtype=np.float32)
    weight = np.asarray(weight, dtype=np.float32)
    bias = np.asarray(bias, dtype=np.float32)
    out, _ = run(H, U, I, weight, bias, trace=False)
    return out
